# revision 1
# baseline (speedup 1.0000x reference)
"""Trainium2 Bass kernel for nn_DensityGrid.

Computes, for a [96,96,96] density grid:
  out_density = 1 - exp(-0.01 * relu(density))
  new_cached  = max(0.8 * density_cached, relu(density))
  field       = maxpool3d(1 - exp(-0.01 * new_cached), k=3, s=1, p=1)
  mask        = field > min(mean(field), 0.01)
  new_field   = largest-connected-component(mask)   (26-connectivity, as a
                288-iteration masked max-dilation in the reference)
  valid       = new_field if step < 500 else old_field

Sharding: z-axis split across 8 NeuronCores (12 planes each + 1-plane halo,
host-padded so no device-side halo exchange is needed).

The connected-component labeling is algebraically short-circuited when the
device proves mask is all-True (min(field) > thr): the masked dilation then
converges to the constant G^3 label within 95 <= 288 iterations, so
new_field == all-True exactly. The device computes per-partition sum/min of
the pooled field; the host checks the condition on those 8*96 scalars. If it
ever fails (never for this workload's data distribution), an exact NumPy
replication of the reference computes new_field instead.
"""

import sys

for _p in ("/opt/trn_rl_repo", "/root/.axon_site/_ro/trn_rl_repo"):
    if _p not in sys.path:
        sys.path.append(_p)

import numpy as np

G = 96
NCORES = 8
ZS = G // NCORES          # 12 interior planes per core
ZH = ZS + 2               # with halo
XP = G + 2                # x padded for pool shifts

_CACHE = {}


def _build_program():
    import concourse.bass as bass
    from concourse import bacc, mybir
    import concourse.tile as tile
    from concourse.masks import make_identity

    f32 = mybir.dt.float32
    Alu = mybir.AluOpType
    Act = mybir.ActivationFunctionType

    nc = bacc.Bacc("TRN2", target_bir_lowering=False, debug=False,
                   num_devices=NCORES)

    d_in = nc.declare_dram_parameter("d", [ZH, G, G], f32, isOutput=False)
    c_in = nc.declare_dram_parameter("c", [ZH, G, G], f32, isOutput=False)
    outd = nc.declare_dram_parameter("outd", [ZS, G, G], f32, isOutput=True)
    outc = nc.declare_dram_parameter("outc", [ZS, G, G], f32, isOutput=True)
    stats = nc.declare_dram_parameter("stats", [G, 2], f32, isOutput=True)

    # DRAM [z,y,x] viewed as [y,z,x] so y sits on partitions.
    d_ap = d_in.ap().rearrange("a b c -> b a c")
    c_ap = c_in.ap().rearrange("a b c -> b a c")
    outd_ap = outd.ap().rearrange("a b c -> b a c")
    outc_ap = outc.ap().rearrange("a b c -> b a c")

    with tile.TileContext(nc) as tc:
        with (
            tc.tile_pool(name="io", bufs=1) as io,
            tc.tile_pool(name="work", bufs=1) as work,
            tc.tile_pool(name="psum", bufs=1, space="PSUM") as psum,
        ):
            t_d = io.tile([G, ZH, G], f32)
            t_c = io.tile([G, ZH, G], f32)
            nc.sync.dma_start(out=t_d[:], in_=d_ap)
            nc.sync.dma_start(out=t_c[:], in_=c_ap)

            ident = work.tile([G, G], f32, tag="ident")
            make_identity(nc, ident[:])

            # relu(d) on all 14 planes (feeds both new_cached and out_density)
            t_rd = work.tile([G, ZH, G], f32, tag="rd")
            nc.vector.tensor_scalar_max(t_rd[:], t_d[:], 0.0)

            # new_cached = max(0.8*c, relu(d))
            t_nc = work.tile([G, ZH, G], f32, tag="nc")
            nc.vector.scalar_tensor_tensor(
                t_nc[:], t_c[:], 0.8, t_rd[:], Alu.mult, Alu.max)
            nc.sync.dma_start(out=outc_ap, in_=t_nc[:, 1:ZS + 1, :])

            # out_density = 1 - exp(-0.01 * relu(d)), interior planes only
            t_e1 = work.tile([G, ZS, G], f32, tag="e1")
            nc.scalar.activation(t_e1[:], t_rd[:, 1:ZS + 1, :], Act.Exp,
                                 scale=-0.01)
            t_od = work.tile([G, ZS, G], f32, tag="od")
            nc.scalar.activation(t_od[:], t_e1[:], Act.Copy, bias=1.0,
                                 scale=-1.0)
            nc.sync.dma_start(out=outd_ap, in_=t_od[:])

            # field0 = 1 - exp(-0.01 * new_cached) on all 14 planes, x-padded
            t_e2 = work.tile([G, ZH, G], f32, tag="e2")
            nc.scalar.activation(t_e2[:], t_nc[:], Act.Exp, scale=-0.01)
            t_f = work.tile([G, ZH, XP], f32, tag="f")
            nc.gpsimd.memset(t_f[:, :, 0:1], 0.0)
            nc.gpsimd.memset(t_f[:, :, XP - 1:XP], 0.0)
            nc.scalar.activation(t_f[:, :, 1:G + 1], t_e2[:], Act.Copy,
                                 bias=1.0, scale=-1.0)

            # pool along x (window 3, zero-padded via guard columns)
            t_t1 = work.tile([G, ZH, G], f32, tag="t1")
            nc.vector.tensor_tensor(
                t_t1[:], t_f[:, :, 0:G], t_f[:, :, 2:G + 2], op=Alu.max)
            t_p1 = work.tile([G, ZH, G], f32, tag="p1")
            nc.vector.tensor_tensor(
                t_p1[:], t_t1[:], t_f[:, :, 1:G + 1], op=Alu.max)

            # pool along z (halo planes close the boundary)
            t_t2 = work.tile([G, ZS, G], f32, tag="t2")
            nc.vector.tensor_tensor(
                t_t2[:], t_p1[:, 0:ZS, :], t_p1[:, 2:ZS + 2, :], op=Alu.max)
            t_p2 = work.tile([G, ZS, G], f32, tag="p2")
            nc.vector.tensor_tensor(
                t_p2[:], t_t2[:], t_p1[:, 1:ZS + 1, :], op=Alu.max)

            # pool along y: transpose each z-plane (y,x)->(x,y) on TensorE
            ps = psum.tile([G, ZS, 128], f32)
            for z in range(ZS):
                nc.tensor.transpose(ps[:, z, 0:G], t_p2[:, z, :], ident[:])
            t_pt = work.tile([G, ZS, G], f32, tag="pt")
            nc.scalar.copy(t_pt[:], ps[:, :, 0:G])

            # w[y] = max(p[y], p[y+1]); v[y] = max(w[y-1], w[y])
            t_w = work.tile([G, ZS, G], f32, tag="w")
            nc.vector.tensor_tensor(
                t_w[:, :, 0:G - 1], t_pt[:, :, 0:G - 1], t_pt[:, :, 1:G],
                op=Alu.max)
            t_v = work.tile([G, ZS, G], f32, tag="v")
            nc.vector.tensor_tensor(
                t_v[:, :, 1:G - 1], t_w[:, :, 0:G - 2], t_w[:, :, 1:G - 1],
                op=Alu.max)
            nc.vector.tensor_copy(t_v[:, :, 0:1], t_w[:, :, 0:1])
            nc.vector.tensor_copy(t_v[:, :, G - 1:G], t_w[:, :, G - 2:G - 1])

            # per-partition stats of pooled field: [sum, min]
            t_stats = work.tile([G, 2], f32, tag="stats")
            nc.vector.tensor_reduce(
                t_stats[:, 0:1], t_v[:], axis=mybir.AxisListType.XY,
                op=Alu.add)
            nc.vector.tensor_reduce(
                t_stats[:, 1:2], t_v[:], axis=mybir.AxisListType.XY,
                op=Alu.min)
            nc.sync.dma_start(out=stats.ap(), in_=t_stats[:])

    nc.compile()
    return nc


def _get_program():
    if "nc" not in _CACHE:
        _CACHE["nc"] = _build_program()
    return _CACHE["nc"]


def _pool1(x, ax):
    pad = [(0, 0)] * 3
    pad[ax] = (1, 1)
    xp = np.pad(x, pad)
    sl = lambda s: tuple(
        slice(s, s + G) if i == ax else slice(None) for i in range(3))
    return np.maximum(np.maximum(xp[sl(0)], xp[sl(1)]), xp[sl(2)])


def _pool3(x):
    return _pool1(_pool1(_pool1(x, 0), 1), 2)


def _numpy_new_field(density, density_cached):
    """Exact NumPy replication of the reference's mask + CCL path."""
    d = np.maximum(density.astype(np.float32), np.float32(0.0))
    ncache = np.maximum(density_cached.astype(np.float32) * np.float32(0.8), d)
    field = _pool3((np.float32(1.0) - np.exp(-np.float32(0.01) * ncache)
                    ).astype(np.float32))
    thr = min(field.mean(dtype=np.float32), np.float32(0.01))
    mask = field > thr
    m = mask.astype(np.float32)
    comp = np.arange(1, G ** 3 + 1, dtype=np.float32).reshape(G, G, G) * m
    for _ in range(3 * G):
        new = _pool3(comp) * m
        if np.array_equal(new, comp):
            break
        comp = new
    labels = comp.astype(np.int32)
    counts = np.zeros(G ** 3 + 1, np.float32)
    np.add.at(counts, labels.ravel(), m.ravel())
    counts[0] = -1.0
    label = np.int32(counts.argmax())
    return labels == label


def kernel(density, density_cached, old_field, step):
    from concourse.bass_utils import run_bass_kernel_spmd

    density = np.ascontiguousarray(np.asarray(density, dtype=np.float32))
    density_cached = np.ascontiguousarray(
        np.asarray(density_cached, dtype=np.float32))
    old_field = np.asarray(old_field).astype(bool)
    step_i = int(np.asarray(step))

    # z-halo padding on host; each core sees planes [12k-1, 12k+13).
    dpad = np.zeros((G + 2, G, G), np.float32)
    cpad = np.zeros((G + 2, G, G), np.float32)
    dpad[1:G + 1] = density
    cpad[1:G + 1] = density_cached
    in_maps = [
        {"d": np.ascontiguousarray(dpad[k * ZS:k * ZS + ZH]),
         "c": np.ascontiguousarray(cpad[k * ZS:k * ZS + ZH])}
        for k in range(NCORES)
    ]

    nc = _get_program()
    res = run_bass_kernel_spmd(nc, in_maps, core_ids=list(range(NCORES)))
    _CACHE["last_results"] = res

    out_density = np.concatenate(
        [res.results[k]["outd"] for k in range(NCORES)], axis=0)
    new_cached = np.concatenate(
        [res.results[k]["outc"] for k in range(NCORES)], axis=0)
    all_stats = np.stack([res.results[k]["stats"] for k in range(NCORES)])
    field_sum = float(all_stats[:, :, 0].sum(dtype=np.float64))
    field_min = float(all_stats[:, :, 1].min())

    mean_field = field_sum / G ** 3
    thr = min(mean_field, 0.01)
    if mean_field >= 0.01 and field_min > thr:
        # mask provably all-True -> labels converge to G^3 everywhere well
        # within the reference's 288 iterations -> new_field is all-True.
        new_field = np.ones((G, G, G), dtype=bool)
    else:
        new_field = _numpy_new_field(density, density_cached)

    valid = new_field if step_i < 500 else old_field
    return (out_density, valid, new_field, new_cached)


# revision 11
# speedup vs baseline: 2.6783x; 2.6783x over previous
"""Trainium2 Bass kernel for nn_DensityGrid.

Reference computation on a [96,96,96] grid:
  out_density = 1 - exp(-0.01 * relu(density))
  new_cached  = max(0.8 * density_cached, relu(density))
  field       = maxpool3d(1 - exp(-0.01 * new_cached), k=3, s=1, p=1)
  mask        = field > min(mean(field), 0.01)
  new_field   = largest connected component of mask (26-conn; reference uses
                a 288-iteration masked max-dilation)
  valid       = new_field if step < 500 else old_field

Sharding: z-axis split across 8 NeuronCores, 12 planes each, processed as two
6-plane chunks pipelined over DMA/ScalarE/VectorE/GpSimd.

Device-side algebra:
  * m = max(0.8*c, d) (one fused scalar_tensor_tensor) feeds BOTH outputs:
    new_cached = max(m, 0) and e = exp(-0.01*m) — a single Exp per chunk.
    (e may exceed 1 when m < 0; that only inflates the conservative stat.)
  * e-domain pooling: field = 1 - e is monotone decreasing in e, so
    maxpool(field) == 1 - minpool(e); the stat pools e directly (bf16).
  * out_density = relu(1 - exp(-0.01*d)) == 1 - exp(-0.01*relu(d)) exactly.
  * CCL short-circuit: mask = field > min(mean(field),0.01); if
    field > 0.01 everywhere the mask is all-True regardless of the mean, and
    the reference's 288-iteration masked max-dilation provably converges to
    the constant G^3 label (grid L-inf diameter 95), making new_field
    all-True exactly. The device reports count(minpool_x(e) >= 0.986); since
    the y/z pools only lower values further, count==0 soundly proves
    pooled e < 0.986 everywhere, i.e. field > 0.0101 > thr (bf16 noise and
    exp-table error < 0.004 are inside the margin). x-windows stay within a
    plane, so no z-halo is needed anywhere. If the check ever fails, an
    exact NumPy replication of the reference computes new_field instead
    (never taken for this workload's data distribution).
"""

import sys

for _p in ("/opt/trn_rl_repo", "/root/.axon_site/_ro/trn_rl_repo"):
    if _p not in sys.path:
        sys.path.append(_p)

import numpy as np

G = 96
NCORES = 8
ZS = G // NCORES          # 12 planes per core
XP = G + 2                # x padded for pool shifts
VMAX = 0.986              # pooled-e acceptance threshold (margin vs 0.99)

_CACHE = {}


def _build_program():
    import concourse.bass as bass
    from concourse import bacc, mybir
    import concourse.tile as tile

    f32 = mybir.dt.float32
    bf16 = mybir.dt.bfloat16
    Alu = mybir.AluOpType
    Act = mybir.ActivationFunctionType

    nc = bacc.Bacc("TRN2", target_bir_lowering=False, debug=False,
                   num_devices=NCORES)

    # Host supplies/consumes [y,z,x] layout so every DMA is contiguous.
    d_in = nc.declare_dram_parameter("d", [G, ZS, G], f32, isOutput=False)
    c_in = nc.declare_dram_parameter("c", [G, ZS, G], f32, isOutput=False)
    outd = nc.declare_dram_parameter("outd", [G, ZS, G], f32, isOutput=True)
    outc = nc.declare_dram_parameter("outc", [G, ZS, G], f32, isOutput=True)
    stats = nc.declare_dram_parameter("stats", [G, 2], f32, isOutput=True)

    d_ap = d_in.ap()
    c_ap = c_in.ap()
    outd_ap = outd.ap()
    outc_ap = outc.ap()

    with tile.TileContext(nc) as tc:
        with (
            tc.tile_pool(name="io", bufs=1) as io,
            tc.tile_pool(name="work", bufs=1) as work,
        ):
            t_stats = work.tile([G, 2], f32, tag="stats")

            ZC = ZS // 2   # planes per chunk
            for ch in range(2):
                zlo = ch * ZC
                t_d = io.tile([G, ZC, G], f32, tag=f"d{ch}")
                t_c = io.tile([G, ZC, G], f32, tag=f"c{ch}")
                nc.sync.dma_start(out=t_d[:], in_=d_ap[:, zlo:zlo + ZC, :])
                nc.sync.dma_start(out=t_c[:], in_=c_ap[:, zlo:zlo + ZC, :])

                # m = max(0.8*c, d); new_cached = max(m, 0); e = exp(-0.01*m)
                t_m = work.tile([G, ZC, G], f32, tag=f"m{ch}")
                nc.vector.scalar_tensor_tensor(
                    t_m[:], t_c[:], 0.8, t_d[:], Alu.mult, Alu.max)
                t_nc = work.tile([G, ZC, G], f32, tag=f"nc{ch}")
                nc.vector.tensor_scalar_max(t_nc[:], t_m[:], 0.0)
                nc.sync.dma_start(out=outc_ap[:, zlo:zlo + ZC, :],
                                  in_=t_nc[:])

                t_f = work.tile([G, ZC, XP], bf16, tag=f"f{ch}")
                nc.gpsimd.memset(t_f[:, :, 0:1], 1.0)
                nc.gpsimd.memset(t_f[:, :, XP - 1:XP], 1.0)
                nc.scalar.activation(t_f[:, :, 1:G + 1], t_m[:], Act.Exp,
                                     scale=-0.01)

                # min-pool along x; y/z pools only lower values, so the
                # stat on the x-pooled tensor upper-bounds the full pool.
                t_t1 = work.tile([G, ZC, G], bf16, tag=f"t1{ch}")
                nc.vector.tensor_tensor(
                    t_t1[:], t_f[:, :, 0:G], t_f[:, :, 2:G + 2], op=Alu.min)
                t_p1 = work.tile([G, ZC, G], bf16, tag=f"p1{ch}")
                nc.vector.tensor_tensor(
                    t_p1[:], t_t1[:], t_f[:, :, 1:G + 1], op=Alu.min)
                t_q = work.tile([G, ZC, G], bf16, tag=f"q{ch}")
                nc.vector.tensor_scalar(
                    t_q[:], t_p1[:], VMAX, None, Alu.is_ge, Alu.add,
                    accum_out=t_stats[:, ch:ch + 1])

                # out_density = relu(1 - exp(-0.01*d))
                t_ed = work.tile([G, ZC, G], f32, tag=f"ed{ch}")
                nc.scalar.activation(t_ed[:], t_d[:], Act.Exp, scale=-0.01)
                t_od = work.tile([G, ZC, G], f32, tag=f"od{ch}")
                nc.scalar.activation(t_od[:], t_ed[:], Act.Relu,
                                     bias=1.0, scale=-1.0)
                nc.sync.dma_start(out=outd_ap[:, zlo:zlo + ZC, :],
                                  in_=t_od[:])
            nc.sync.dma_start(out=stats.ap(), in_=t_stats[:])

    nc.compile()
    return nc


def _get_program():
    if "nc" not in _CACHE:
        _CACHE["nc"] = _build_program()
    return _CACHE["nc"]


def _pool1(x, ax):
    pad = [(0, 0)] * 3
    pad[ax] = (1, 1)
    xp = np.pad(x, pad)
    sl = lambda s: tuple(
        slice(s, s + G) if i == ax else slice(None) for i in range(3))
    return np.maximum(np.maximum(xp[sl(0)], xp[sl(1)]), xp[sl(2)])


def _pool3(x):
    return _pool1(_pool1(_pool1(x, 0), 1), 2)


def _numpy_new_field(density, density_cached):
    """Exact NumPy replication of the reference's mask + CCL path."""
    d = np.maximum(density.astype(np.float32), np.float32(0.0))
    ncache = np.maximum(density_cached.astype(np.float32) * np.float32(0.8), d)
    field = _pool3((np.float32(1.0) - np.exp(-np.float32(0.01) * ncache)
                    ).astype(np.float32))
    thr = min(field.mean(dtype=np.float32), np.float32(0.01))
    mask = field > thr
    m = mask.astype(np.float32)
    comp = np.arange(1, G ** 3 + 1, dtype=np.float32).reshape(G, G, G) * m
    for _ in range(3 * G):
        new = _pool3(comp) * m
        if np.array_equal(new, comp):
            break
        comp = new
    labels = comp.astype(np.int32)
    counts = np.zeros(G ** 3 + 1, np.float32)
    np.add.at(counts, labels.ravel(), m.ravel())
    counts[0] = -1.0
    label = np.int32(counts.argmax())
    return labels == label


def kernel(density, density_cached, old_field, step):
    from concourse.bass_utils import run_bass_kernel_spmd

    density = np.ascontiguousarray(np.asarray(density, dtype=np.float32))
    density_cached = np.ascontiguousarray(
        np.asarray(density_cached, dtype=np.float32))
    old_field = np.asarray(old_field).astype(bool)
    step_i = int(np.asarray(step))

    in_maps = [
        {"d": np.ascontiguousarray(
            density[k * ZS:(k + 1) * ZS].transpose(1, 0, 2)),
         "c": np.ascontiguousarray(
            density_cached[k * ZS:(k + 1) * ZS].transpose(1, 0, 2))}
        for k in range(NCORES)
    ]

    nc = _get_program()
    res = run_bass_kernel_spmd(nc, in_maps, core_ids=list(range(NCORES)))
    _CACHE["last_results"] = res

    out_density = np.concatenate(
        [res.results[k]["outd"].transpose(1, 0, 2) for k in range(NCORES)],
        axis=0)
    new_cached = np.concatenate(
        [res.results[k]["outc"].transpose(1, 0, 2) for k in range(NCORES)],
        axis=0)
    over_count = float(
        sum(res.results[k]["stats"].sum(dtype=np.float64)
            for k in range(NCORES)))

    if over_count == 0.0:
        # pooled e < 0.986 everywhere -> field > 0.0101 > min(mean, 0.01)
        # -> mask all-True -> CCL converges to all-True exactly.
        new_field = np.ones((G, G, G), dtype=bool)
    else:
        new_field = _numpy_new_field(density, density_cached)

    valid = new_field if step_i < 500 else old_field
    return (out_density, valid, new_field, new_cached)


# revision 17
# speedup vs baseline: 2.6921x; 1.0052x over previous
"""Trainium2 Bass kernel for nn_DensityGrid.

Reference computation on a [96,96,96] float32 grid:
  out_density = 1 - exp(-0.01 * relu(density))
  new_cached  = max(0.8 * density_cached, relu(density))
  field       = maxpool3d(1 - exp(-0.01 * new_cached), k=3, s=1, p=1)
  mask        = field > min(mean(field), 0.01)
  new_field   = largest connected component of mask (26-connectivity; the
                reference runs a 288-iteration masked max-dilation)
  valid       = new_field if step < 500 else old_field

Sharding: z-axis split across 8 NeuronCores, 12 planes per core, processed
as two 6-plane chunks so DMA / ScalarE / VectorE overlap. Host passes shards
pre-permuted to [y,z,x] so every DMA is a contiguous-row transfer.

Device-side algebra (per core):
  * m = max(0.8*c, d) via one fused scalar_tensor_tensor; new_cached is then
    just max(m, 0) and out_density = relu(1 - exp(-0.01*d)) (one Exp + one
    fused affine-Relu activation) == 1 - exp(-0.01*relu(d)) exactly.
  * CCL short-circuit: mask = field > min(mean(field), 0.01) and
    min(mean,0.01) <= 0.01, so `field > 0.01 everywhere` makes the mask
    all-True regardless of the mean; the reference's masked max-dilation then
    provably converges to the constant G^3 label inside its 288 iterations
    (grid L-inf diameter is 95), i.e. new_field is exactly all-True.
  * The all-True proof is computed in m-domain, f32-exact, with ONE fused
    instruction per chunk (tensor_tensor_reduce):
        stat = min over shard of max(m[..., 2i], m[..., 2i+1])
    Every voxel's 3x3x3 pool window contains its own disjoint x-pair, so
    maxpool3d(m') >= pairmax everywhere (m' = relu(m) = new_cached, and the
    pair values are positive whenever the check passes). Host condition
    stat > 1.006 > -100*ln(0.99) then guarantees
    field = 1 - exp(-0.01*maxpool(new_cached)) > 0.01 everywhere even after
    the reference's f32 exp rounding. If the check fails, an exact NumPy
    replication of the reference computes new_field (not taken for this
    workload's data distribution: actual stat ~ 3.5).
"""

import sys

for _p in ("/opt/trn_rl_repo", "/root/.axon_site/_ro/trn_rl_repo"):
    if _p not in sys.path:
        sys.path.append(_p)

import numpy as np

G = 96
NCORES = 8
ZS = G // NCORES          # 12 planes per core
MTHR = 1.006              # m-domain acceptance threshold (-100*ln(0.99)=1.00503)

_CACHE = {}


def _build_program():
    import concourse.bass as bass
    from concourse import bacc, mybir
    import concourse.tile as tile

    f32 = mybir.dt.float32
    Alu = mybir.AluOpType
    Act = mybir.ActivationFunctionType

    nc = bacc.Bacc("TRN2", target_bir_lowering=False, debug=False,
                   num_devices=NCORES)

    # Host supplies/consumes [y,z,x] layout so every DMA is contiguous.
    d_in = nc.declare_dram_parameter("d", [G, ZS, G], f32, isOutput=False)
    c_in = nc.declare_dram_parameter("c", [G, ZS, G], f32, isOutput=False)
    outd = nc.declare_dram_parameter("outd", [G, ZS, G], f32, isOutput=True)
    outc = nc.declare_dram_parameter("outc", [G, ZS, G], f32, isOutput=True)
    stats = nc.declare_dram_parameter("stats", [G, 2], f32, isOutput=True)

    d_ap = d_in.ap()
    c_ap = c_in.ap()
    outd_ap = outd.ap()
    outc_ap = outc.ap()

    with tile.TileContext(nc) as tc:
        with (
            tc.tile_pool(name="io", bufs=1) as io,
            tc.tile_pool(name="work", bufs=1) as work,
        ):
            t_stats = work.tile([G, 2], f32, tag="stats")

            ZC = ZS // 2   # planes per chunk
            tiles = []
            # stat chain first (higher scheduler priority) ...
            for ch in range(2):
                zlo = ch * ZC
                t_d = io.tile([G, ZC, G], f32, tag=f"d{ch}")
                t_c = io.tile([G, ZC, G], f32, tag=f"c{ch}")
                nc.sync.dma_start(out=t_d[:], in_=d_ap[:, zlo:zlo + ZC, :])
                nc.sync.dma_start(out=t_c[:], in_=c_ap[:, zlo:zlo + ZC, :])

                # m = max(0.8*c, d)
                t_m = work.tile([G, ZC, G], f32, tag=f"m{ch}")
                nc.vector.scalar_tensor_tensor(
                    t_m[:], t_c[:], 0.8, t_d[:], Alu.mult, Alu.max)
                # stat: min over the shard of sliding-pair maxes of m.
                # Every voxel's 3x3x3 pool window contains the x-pair
                # (x, x+1) (or (x-1, x) at the edge), so
                # min(pairmax) > T proves maxpool3d(m-field) clears T
                # everywhere; f32-exact, no exp needed on the stat path.
                t_r1 = work.tile([G, ZC, G - 1], f32, tag=f"r1{ch}")
                nc.vector.tensor_tensor(
                    t_r1[:], t_m[:, :, 0:G - 1], t_m[:, :, 1:G],
                    op=Alu.max)
                nc.vector.tensor_reduce(
                    t_stats[:, ch:ch + 1], t_r1[:],
                    axis=mybir.AxisListType.XY, op=Alu.min)
                tiles.append((zlo, t_d, t_c, t_m))
            nc.sync.dma_start(out=stats.ap(), in_=t_stats[:])

            # ... exact output paths second (fill engine slack)
            for ch in range(2):
                zlo, t_d, t_c, t_m = tiles[ch]
                t_nc = work.tile([G, ZC, G], f32, tag=f"nc{ch}")
                nc.vector.tensor_scalar_max(t_nc[:], t_m[:], 0.0)
                nc.sync.dma_start(out=outc_ap[:, zlo:zlo + ZC, :],
                                  in_=t_nc[:])
                # out_density = relu(1 - exp(-0.01*d))
                t_ed = work.tile([G, ZC, G], f32, tag=f"ed{ch}")
                nc.scalar.activation(t_ed[:], t_d[:], Act.Exp, scale=-0.01)
                t_od = work.tile([G, ZC, G], f32, tag=f"od{ch}")
                nc.scalar.activation(t_od[:], t_ed[:], Act.Relu,
                                     bias=1.0, scale=-1.0)
                nc.sync.dma_start(out=outd_ap[:, zlo:zlo + ZC, :],
                                  in_=t_od[:])

    nc.compile()
    return nc


def _get_program():
    if "nc" not in _CACHE:
        _CACHE["nc"] = _build_program()
    return _CACHE["nc"]


def _pool1(x, ax):
    pad = [(0, 0)] * 3
    pad[ax] = (1, 1)
    xp = np.pad(x, pad)
    sl = lambda s: tuple(
        slice(s, s + G) if i == ax else slice(None) for i in range(3))
    return np.maximum(np.maximum(xp[sl(0)], xp[sl(1)]), xp[sl(2)])


def _pool3(x):
    return _pool1(_pool1(_pool1(x, 0), 1), 2)


def _numpy_new_field(density, density_cached):
    """Exact NumPy replication of the reference's mask + CCL path."""
    d = np.maximum(density.astype(np.float32), np.float32(0.0))
    ncache = np.maximum(density_cached.astype(np.float32) * np.float32(0.8), d)
    field = _pool3((np.float32(1.0) - np.exp(-np.float32(0.01) * ncache)
                    ).astype(np.float32))
    thr = min(field.mean(dtype=np.float32), np.float32(0.01))
    mask = field > thr
    m = mask.astype(np.float32)
    comp = np.arange(1, G ** 3 + 1, dtype=np.float32).reshape(G, G, G) * m
    for _ in range(3 * G):
        new = _pool3(comp) * m
        if np.array_equal(new, comp):
            break
        comp = new
    labels = comp.astype(np.int32)
    counts = np.zeros(G ** 3 + 1, np.float32)
    np.add.at(counts, labels.ravel(), m.ravel())
    counts[0] = -1.0
    label = np.int32(counts.argmax())
    return labels == label


def kernel(density, density_cached, old_field, step):
    from concourse.bass_utils import run_bass_kernel_spmd

    density = np.ascontiguousarray(np.asarray(density, dtype=np.float32))
    density_cached = np.ascontiguousarray(
        np.asarray(density_cached, dtype=np.float32))
    old_field = np.asarray(old_field).astype(bool)
    step_i = int(np.asarray(step))

    in_maps = [
        {"d": np.ascontiguousarray(
            density[k * ZS:(k + 1) * ZS].transpose(1, 0, 2)),
         "c": np.ascontiguousarray(
            density_cached[k * ZS:(k + 1) * ZS].transpose(1, 0, 2))}
        for k in range(NCORES)
    ]

    nc = _get_program()
    res = run_bass_kernel_spmd(nc, in_maps, core_ids=list(range(NCORES)))
    _CACHE["last_results"] = res

    out_density = np.concatenate(
        [res.results[k]["outd"].transpose(1, 0, 2) for k in range(NCORES)],
        axis=0)
    new_cached = np.concatenate(
        [res.results[k]["outc"].transpose(1, 0, 2) for k in range(NCORES)],
        axis=0)
    stat_min = float(
        min(res.results[k]["stats"].min() for k in range(NCORES)))

    if stat_min > MTHR:
        # every voxel has an in-window pair with m > MTHR > -100*ln(0.99),
        # so field > 0.01 >= min(mean, 0.01) everywhere -> mask all-True
        # -> the reference CCL converges to all-True exactly.
        new_field = np.ones((G, G, G), dtype=bool)
    else:
        new_field = _numpy_new_field(density, density_cached)

    valid = new_field if step_i < 500 else old_field
    return (out_density, valid, new_field, new_cached)
